# revision 3
# baseline (speedup 1.0000x reference)
"""MoE grouped-linear (ragged matmul + bias) on 8 TRN2 NeuronCores.

Expert-parallel sharding: core e computes tokens of expert e:
    out_e = X_e[cap, 2048] @ W_e[2048, 8192] + bias
Tokens are pre-sorted by expert (contiguous groups), so the "all-to-all"
is a free host-side slice/concat. No on-device collectives.

Per-core kernel: float32r (TF32-like multiply, fp32 accumulate) with
self-loading matmuls. Loop order is n-chunk-outer / m-inner so each
[128, 16, 512] W k-slice is DMA'd once and reused by all 8 m-tiles;
X^T and bias stay SBUF-resident. Two accumulation chains run in
interleaved issue order on separate PSUM banks (ilv=2) so chain
start/stop and eviction never serialize against the tensor engine.
Bias is fused into PSUM eviction on the vector engine; output DMA uses
the Activation-engine queue so W prefetch on the sync queue is never
blocked behind output writes.

Measured on TRN2 via reps-slope: ~540 us/core steady-state vs ~566-571
us for the previous kernel in the same back-to-back session (absolute
numbers drift +/-10% with host load; the ordering was stable across
sessions). This sits at the hardware floor for this decomposition:
each of the 2048 matmuls pays a serial 128-row stationary load plus a
512-row stream (2048 x (53 + 213) ns ~= 545 us). Weight-load reuse
across matmuls is impossible: the PE weight buffer acts as a FIFO
(verified — deleting paired Ldweights corrupts results), and bf16
split Ldweights+Matmult pairs are ~50 ns/pair slower than fp32r
self-loading. fp8 halves stream time but fails the 2e-2 gate
(absmax ~0.2 vs budget 0.11).
"""

import numpy as np

E, IN, OUT = 8, 2048, 8192
P = 128
NT = 512
ILV = 2

_cache = {}


def _build(cap, reps=1, nt=NT, ilv=ILV):
    import contextlib

    import concourse.mybir as mybir
    import concourse.tile as tile
    from concourse import bacc

    mm_dt = mybir.dt.float32r
    KT = IN // P            # 16 k-tiles
    MT = cap // P           # m-tiles per core
    NQ = OUT // nt          # 16 n-chunks
    assert MT % ilv == 0

    nc = bacc.Bacc(None, target_bir_lowering=False, debug=False)
    with tile.TileContext(nc) as tc:
        with tc.tile_pool(name="dram", bufs=1, space="DRAM") as dram:
            # xt[mi, p, k, j] = X[mi*P + j, k*P + p]
            xt_d = dram.tile((MT, P, KT, P), mm_dt, kind="ExternalInput")
            # w[k, p, n] = W[k*P + p, n]  (plain reshape of W, no transpose)
            w_d = dram.tile((KT, P, OUT), mm_dt, kind="ExternalInput")
            bias_d = dram.tile((P, OUT), mybir.dt.float32,
                               kind="ExternalInput")
            out_d = dram.tile((P, MT, OUT), mybir.dt.float32,
                              kind="ExternalOutput")

            with tc.tile_pool(name="resident", bufs=1) as res_pool, \
                 tc.tile_pool(name="wsl", bufs=2) as w_pool, \
                 tc.tile_pool(name="evict", bufs=2 * ilv) as o_pool, \
                 tc.tile_pool(name="acc", bufs=2 * ilv,
                              space="PSUM") as ps_pool:
                loop = tc.For_i(0, reps, 1) if reps > 1 else (
                    contextlib.nullcontext())
                with loop:
                    xt_sb = [res_pool.tile([P, KT, P], mm_dt, tag=f"xt{mi}",
                                           name=f"xt_sb{mi}")
                             for mi in range(MT)]
                    bias_sb = res_pool.tile([P, OUT], mybir.dt.float32)
                    nc.gpsimd.dma_start(xt_sb[0][:], xt_d[0])
                    nc.gpsimd.dma_start(xt_sb[1][:], xt_d[1])
                    nc.gpsimd.dma_start(bias_sb[:], bias_d[:])
                    for mi in range(2, MT):
                        nc.gpsimd.dma_start(xt_sb[mi][:], xt_d[mi])

                    # All W slice DMAs upfront on the sync queue; pool slot
                    # reuse (bufs=2 per k-tag) gates chunk q+2 behind chunk
                    # q's last matmul reader.
                    w_sb = {}
                    for q in range(NQ):
                        for k in range(KT):
                            t = w_pool.tile([P, nt], mm_dt, tag=f"w{k}",
                                            name=f"w_sb_{q}_{k}")
                            nc.sync.dma_start(
                                t[:], w_d[k, :, q * nt:(q + 1) * nt])
                            w_sb[q, k] = t

                    for q in range(NQ):
                        for mi0 in range(0, MT, ilv):
                            mis = range(mi0, mi0 + ilv)
                            ps = {mi: ps_pool.tile(
                                      [P, nt], mybir.dt.float32,
                                      tag=f"acc{mi - mi0}",
                                      name=f"ps_{q}_{mi}")
                                  for mi in mis}
                            for k in range(KT):
                                for mi in mis:
                                    nc.tensor.matmul(
                                        ps[mi][:],
                                        lhsT=xt_sb[mi][:, k, :],
                                        rhs=w_sb[q, k][:],
                                        start=(k == 0),
                                        stop=(k == KT - 1),
                                    )
                            for mi in mis:
                                o_sb = o_pool.tile(
                                    [P, nt], mybir.dt.float32,
                                    tag=f"o{mi - mi0}",
                                    name=f"o_{q}_{mi}")
                                nc.vector.tensor_add(
                                    out=o_sb[:], in0=ps[mi][:],
                                    in1=bias_sb[:, q * nt:(q + 1) * nt])
                                nc.scalar.dma_start(
                                    out_d[:, mi, q * nt:(q + 1) * nt],
                                    o_sb[:])
    nc.compile()
    names = dict(xt=xt_d.name, w=w_d.name, bias=bias_d.name, out=out_d.name)
    return nc, names


def _get(cap, reps=1, nt=NT, ilv=ILV):
    key = (cap, reps, nt, ilv)
    if key not in _cache:
        _cache[key] = _build(cap, reps=reps, nt=nt, ilv=ilv)
    return _cache[key]


def kernel(inputs, weight, group_sizes, bias):
    from concourse.bass_utils import run_bass_kernel_spmd

    M = inputs.shape[0]
    gs = np.asarray(group_sizes, dtype=np.int64)
    # per-token expert id exactly as the reference's jnp.repeat(...,
    # total_repeat_length=M): truncate or pad with the last expert id
    ids = np.repeat(np.arange(E), gs)
    ids = ids[:M] if len(ids) >= M else np.concatenate(
        [ids, np.full(M - len(ids), E - 1)])
    counts = np.bincount(ids, minlength=E)
    starts = np.concatenate([[0], np.cumsum(counts)])[:E]

    cap = max(2 * P, int(-(-counts.max() // (2 * P)) * 2 * P))
    nc, names = _get(cap)

    x = np.ascontiguousarray(inputs, dtype=np.float32)
    w = np.asarray(weight, dtype=np.float32)
    bias_rep = np.ascontiguousarray(
        np.broadcast_to(np.asarray(bias, np.float32), (P, OUT)))

    in_maps = []
    for e in range(E):
        xe = x[starts[e]:starts[e] + counts[e]]
        if xe.shape[0] < cap:
            xe = np.concatenate(
                [xe, np.zeros((cap - xe.shape[0], IN), np.float32)])
        # [cap, IN] -> (MT, P, KT, P): xt[mi, p, k, j] = X[mi*P+j, k*P+p]
        xt = np.ascontiguousarray(
            xe.reshape(cap // P, P, IN // P, P).transpose(0, 3, 2, 1))
        # [IN, OUT] -> (KT, P, OUT): contiguous reshape, no copy
        we = np.ascontiguousarray(w[e].reshape(IN // P, P, OUT))
        in_maps.append({names["xt"]: xt, names["w"]: we,
                        names["bias"]: bias_rep})

    res = run_bass_kernel_spmd(nc, in_maps, core_ids=list(range(E)))
    out = np.empty((M, OUT), dtype=np.float32)
    for e in range(E):
        oe = res.results[e][names["out"]]          # (P, cap//P, OUT)
        oe = oe.transpose(1, 0, 2).reshape(cap, OUT)
        out[starts[e]:starts[e] + counts[e]] = oe[:counts[e]]
    return out
